# revision 1
# baseline (speedup 1.0000x reference)
"""Trainium2 Bass kernel for nn_DifferentialMaxtree (N = 4M tree nodes).

Pipeline (8-way data-parallel over tree nodes, one shard per NeuronCore):
  A) linear/sigmoid head over the 15 raw attributes -> contrib = diff * score
     (the memory-dominant stage: streams the 251MB attribute matrix)
  B) tree filter out[i] = sum of contrib over i's ancestor chain, computed
     WITHOUT pointer chasing via a DFS-interval identity: relabel nodes in
     DFS preorder (host index prep); the subtree of node t is the contiguous
     interval [t, end_t], so
         out[i] = P1[i] - P2[i],
         P1 = prefix-sum(contrib),
         P2[i] = R[i - depth(i) - 1],
         R = prefix-sum of contrib permuted into end-sorted (postorder) order.
     The prefix sums (the O(N) value computation) run on device as streaming
     scans with cross-partition offsets folded in via a triangular matmul.

TRN2's DMA engines only support block-granular indirect addressing (one
latched offset per contiguous descriptor run), so the two data-dependent
permutations (contrib -> postorder and the R sampling at i-depth(i)-1) are
applied on the host between the two device launches, as part of the
shard/unshard glue; they are pure index-space reshuffles with no arithmetic.
"""

import math
import numpy as np

N = 4194304
H = W = 2048
NCORES = 8
P = 128
S = N // NCORES  # 524288 nodes per core
F = S // P  # 4096 free elems per partition
EPS = 1e-10

KT = 512  # stage-A rows per partition per tile
NT = F // KT  # stage-A tiles



# ---- inlined compat: this walrus build rejects >1 semaphore wait per
# instruction ("Too many sync wait commands"); split extras onto nops ----
def _split_excess_waits(nc, max_waits=1):
    from concourse import mybir

    counter = 0
    for f in nc.m.functions:
        for bb in f.blocks:
            il = bb.instructions
            i = 0
            while i < len(il):
                inst = il[i]
                si = inst.sync_info
                if si is not None and len(si.on_wait) > max_waits:
                    waits = list(si.on_wait)
                    rest, keep = waits[:-max_waits], waits[-max_waits:]
                    pre = []
                    for j in range(0, len(rest), max_waits):
                        nop = mybir.InstNoOp(
                            name=f"I-waitsplit-{counter}", ins=[], outs=[]
                        )
                        nop.engine = inst.engine
                        nop.sync_info = mybir.SyncInfo(
                            on_wait=list(rest[j : j + max_waits]), on_update=[]
                        )
                        counter += 1
                        pre.append(nop)
                    inst.sync_info = mybir.SyncInfo(
                        on_wait=keep, on_update=list(si.on_update)
                    )
                    for k, p_ in enumerate(pre):
                        il.insert(i + k, p_)
                    i += len(pre)
                i += 1
    return counter


def _host_prep(parent):
    """DFS relabeling, interval ends, postorder rank and sample positions."""
    import scipy.sparse as sp
    from scipy.sparse.csgraph import depth_first_order

    parent = np.asarray(parent).astype(np.int64)
    idx = np.arange(1, N, dtype=np.int64)
    g = sp.csr_matrix((np.ones(N - 1, np.int8), (parent[1:], idx)), shape=(N, N))
    order = np.asarray(
        depth_first_order(g, 0, directed=True, return_predecessors=False),
        dtype=np.int64,
    )
    assert order.shape[0] == N, f"tree not rooted/connected: {order.shape}"

    # depth (number of proper ancestors) via pointer doubling
    SENT = N
    p = np.concatenate([parent, [SENT]])
    p[0] = SENT
    chains = []
    pk = p.copy()
    while not np.all(pk[:N] == SENT):
        chains.append(pk.copy())
        pk = pk[pk]
        pk[SENT] = SENT
    depth = np.zeros(N + 1, np.int64)
    cur = np.arange(N + 1)
    for k in range(len(chains) - 1, -1, -1):
        anc = chains[k][cur]
        mask = anc != SENT
        depth[mask] += 1 << k
        cur = np.where(mask, anc, cur)
    d_old = depth[:N]

    # subtree sizes: accumulate child -> parent, deepest level first
    size = np.ones(N, np.int64)
    dorder = np.argsort(d_old, kind="stable")
    maxd = int(d_old.max())
    dstarts = np.searchsorted(d_old[dorder], np.arange(maxd + 2))
    for lev in range(maxd, 0, -1):
        nodes = dorder[dstarts[lev] : dstarts[lev + 1]]
        np.add.at(size, parent[nodes], size[nodes])

    end_new = np.arange(N, dtype=np.int64) + size[order] - 1
    d_new = d_old[order]
    sigma = np.argsort(end_new, kind="stable")  # rank r -> source node t
    q = np.arange(N, dtype=np.int64) - d_new - 1  # P2 sample position (rho-1)
    return {"order": order, "sigma": sigma, "q": q}


def _build_stage_a(w, b, repeat=1):
    """Program A: attribute head -> contrib, per core shard."""
    from concourse import bass, mybir
    import concourse.tile as tile

    DT = mybir.dt.float32
    AF = mybir.ActivationFunctionType
    OP = mybir.AluOpType

    w = [float(x) for x in w]
    b = float(b)

    nc = bass.Bass()
    attr_d = nc.declare_dram_parameter("attr", [S * 15], DT, isOutput=False)
    diff_d = nc.declare_dram_parameter("diff", [S], DT, isOutput=False)
    con_d = nc.declare_dram_parameter("contrib", [S], DT, isOutput=True)

    # ACT biases must exist as const APs
    for _cv in (EPS, math.pi / 2):
        _ct = nc.alloc_sbuf_tensor(f"const-f32-{_cv}", [P, 1], DT)
        nc.gpsimd.memset(_ct.ap(), _cv)
        nc.const_aps.aps[(DT, _cv)] = _ct.ap()
    nc.all_engine_barrier()

    with tile.TileContext(nc) as tc:
        with tc.tile_pool(name="sbuf", bufs=2) as pool:
            for _rep in range(repeat):
                attr_v = attr_d[:].rearrange("(p x) -> p x", p=P)
                diff_v = diff_d[:].rearrange("(p f) -> p f", p=P)
                con_v = con_d[:].rearrange("(p f) -> p f", p=P)
                for t in range(NT):
                    K = KT
                    at = pool.tile([P, K * 15], DT, tag="at")
                    nc.sync.dma_start(
                        out=at[:], in_=attr_v[:, t * K * 15 : (t + 1) * K * 15]
                    )
                    dt_ = pool.tile([P, K], DT, tag="dt")
                    nc.sync.dma_start(
                        out=dt_[:], in_=diff_v[:, t * K : (t + 1) * K]
                    )
                    # feature-major layout: column j is contiguous [P, K]
                    def col(j):
                        return at[:, j * K : (j + 1) * K]

                    feat = pool.tile([P, 12 * K], DT, tag="feat")
                    nc.scalar.activation(
                        feat[:, 0 : 9 * K], at[:, 6 * K : 15 * K], AF.Ln,
                        bias=EPS,
                    )
                    # range-reduce angles to [0, 2pi) before the Sin LUT
                    # (raw args reach ~11.6 rad where the LUT loses accuracy);
                    # floor(x/2pi) via truncating f32->i32 cast (x >= 0)
                    tq = pool.tile([P, K], DT, tag="tq")
                    nc.vector.tensor_scalar_mul(
                        tq[:], col(5), 1.0 / (2 * math.pi)
                    )
                    tqi = pool.tile([P, K], mybir.dt.int32, tag="tqi")
                    nc.vector.tensor_copy(out=tqi[:], in_=tq[:])
                    nc.vector.tensor_copy(out=tq[:], in_=tqi[:])
                    ang = pool.tile([P, K], DT, tag="ang")
                    nc.vector.scalar_tensor_tensor(
                        out=ang[:], in0=tq[:], scalar=-2 * math.pi,
                        in1=col(5), op0=OP.mult, op1=OP.add,
                    )
                    nc.scalar.activation(
                        feat[:, 9 * K : 10 * K], ang[:], AF.Sin,
                        bias=math.pi / 2,
                    )
                    nc.scalar.activation(
                        feat[:, 10 * K : 11 * K], ang[:], AF.Sin
                    )
                    sq7 = pool.tile([P, K], DT, tag="sq7")
                    sq6 = pool.tile([P, K], DT, tag="sq6")
                    nc.scalar.activation(sq7[:], col(7), AF.Sqrt)
                    nc.scalar.activation(sq6[:], col(6), AF.Sqrt)
                    nc.vector.tensor_scalar_add(sq6[:], sq6[:], EPS)
                    nc.vector.reciprocal(sq6[:], sq6[:])
                    nc.vector.tensor_tensor(
                        out=feat[:, 11 * K : 12 * K], in0=sq7[:], in1=sq6[:],
                        op=OP.mult,
                    )
                    y = pool.tile([P, K], DT, tag="y")
                    nc.scalar.activation(
                        y[:], col(0), AF.Copy, bias=b, scale=w[0]
                    )
                    y1 = pool.tile([P, K], DT, tag="y1")
                    nc.vector.tensor_scalar(
                        out=y1[:], in0=col(1), scalar1=w[1], scalar2=None,
                        op0=OP.mult,
                    )
                    # two independent accumulator chains (y even, y1 odd)
                    terms = [("c", j) for j in range(2, 5)] + [
                        ("f", j) for j in range(12)
                    ]
                    for n, (kind, j) in enumerate(terms):
                        src = (
                            col(j)
                            if kind == "c"
                            else feat[:, j * K : (j + 1) * K]
                        )
                        wt = w[j] if kind == "c" else w[5 + j]
                        acc = y if n % 2 == 0 else y1
                        nc.vector.scalar_tensor_tensor(
                            out=acc[:], in0=src, scalar=wt, in1=acc[:],
                            op0=OP.mult, op1=OP.add,
                        )
                    nc.vector.tensor_tensor(
                        out=y[:], in0=y[:], in1=y1[:], op=OP.add
                    )
                    sc = pool.tile([P, K], DT, tag="sc")
                    nc.scalar.activation(sc[:], y[:], AF.Sigmoid)
                    ct = pool.tile([P, K], DT, tag="ct")
                    nc.vector.tensor_tensor(
                        out=ct[:], in0=sc[:], in1=dt_[:], op=OP.mult
                    )
                    nc.sync.dma_start(
                        out=con_v[:, t * K : (t + 1) * K], in_=ct[:]
                    )

    _split_excess_waits(nc)
    return nc


def _build_scans(repeat=1):
    """Program B: prefix scans of contrib (P1) and postorder contrib (R),
    with cross-partition offsets folded in via a triangular matmul."""
    from concourse import bass, mybir
    import concourse.tile as tile

    DT = mybir.dt.float32
    OP = mybir.AluOpType

    nc = bass.Bass()
    con_d = nc.declare_dram_parameter("contrib", [S], DT, isOutput=False)
    cs_d = nc.declare_dram_parameter("cs", [S], DT, isOutput=False)
    triu_d = nc.declare_dram_parameter("triu", [P, P], DT, isOutput=False)
    p1_d = nc.declare_dram_parameter("p1a", [S], DT, isOutput=True)
    ra_d = nc.declare_dram_parameter("ra", [S], DT, isOutput=True)

    with tile.TileContext(nc) as tc:
        with (
            tc.tile_pool(name="sbuf", bufs=2) as pool,
            tc.tile_pool(name="perm", bufs=1) as perm,
            tc.tile_pool(name="psum", bufs=2, space="PSUM") as psum,
        ):
            for _rep in range(repeat):
                con = perm.tile([P, F], DT)
                nc.sync.dma_start(
                    out=con[:], in_=con_d[:].rearrange("(p f) -> p f", p=P)
                )
                cs = perm.tile([P, F], DT)
                nc.sync.dma_start(
                    out=cs[:], in_=cs_d[:].rearrange("(p f) -> p f", p=P)
                )
                zt = perm.tile([P, F], DT)
                nc.vector.memset(zt[:], 0.0)

                # single-pass scans: per-instruction dispatch overhead
                # dominates in this environment (~tens of us/instruction),
                # and segmenting was measured to not improve accuracy
                p1 = perm.tile([P, F], DT)
                nc.vector.tensor_tensor_scan(
                    out=p1[:], data0=con[:], data1=zt[:], initial=0.0,
                    op0=OP.add, op1=OP.add,
                )
                rsc = perm.tile([P, F], DT)
                nc.vector.tensor_tensor_scan(
                    out=rsc[:], data0=cs[:], data1=zt[:], initial=0.0,
                    op0=OP.add, op1=OP.add,
                )
                triu = perm.tile([P, P], DT)
                nc.sync.dma_start(out=triu[:], in_=triu_d[:])
                tots = pool.tile([P, 2], DT)
                nc.vector.tensor_copy(out=tots[:, 0:1], in_=p1[:, F - 1 : F])
                nc.vector.tensor_copy(out=tots[:, 1:2], in_=rsc[:, F - 1 : F])
                po = psum.tile([P, 2], DT)
                nc.tensor.matmul(
                    out=po[:], lhsT=triu[:], rhs=tots[:], start=True, stop=True
                )
                pos = pool.tile([P, 2], DT)
                nc.vector.tensor_copy(out=pos[:], in_=po[:])
                nc.vector.tensor_scalar(
                    out=p1[:], in0=p1[:], scalar1=pos[:, 0:1], scalar2=None,
                    op0=OP.add,
                )
                nc.vector.tensor_scalar(
                    out=rsc[:], in0=rsc[:], scalar1=pos[:, 1:2], scalar2=None,
                    op0=OP.add,
                )
                nc.sync.dma_start(
                    out=p1_d[:].rearrange("(p f) -> p f", p=P), in_=p1[:]
                )
                nc.sync.dma_start(
                    out=ra_d[:].rearrange("(p f) -> p f", p=P), in_=rsc[:]
                )

    _split_excess_waits(nc)
    return nc


def _prepare_inputs(maxtree_parent, maxtree_diff, attributes):
    diff = np.asarray(maxtree_diff, dtype=np.float32)
    attrs = np.ascontiguousarray(np.asarray(attributes, dtype=np.float32))
    prep = _host_prep(maxtree_parent)
    order = prep["order"]
    attr_p = attrs[order]
    diff_p = diff[order]
    # feature-major tile layout: within each (partition, tile) chunk of KT
    # rows, store column-major [15, KT] so all device reads are unit-stride
    in_maps_a = []
    for c in range(NCORES):
        a = attr_p[c * S : (c + 1) * S].reshape(P, NT, KT, 15)
        a = np.ascontiguousarray(a.transpose(0, 1, 3, 2))  # [P, NT, 15, KT]
        in_maps_a.append(
            {"attr": a.reshape(-1), "diff": diff_p[c * S : (c + 1) * S]}
        )
    return in_maps_a, prep


def _run_device(in_maps_a, prep, w, b, repeat=1, progs=None):
    """Run both device programs; host applies the index permutations between
    them.  Returns (out_new, progs) where progs can be reused for re-runs."""
    from concourse.bass_utils import run_bass_kernel_spmd

    cores = list(range(NCORES))
    if progs is None:
        progs = (_build_stage_a(w, b, repeat), _build_scans(repeat))
    nc_a, nc_b = progs

    res_a = run_bass_kernel_spmd(nc_a, in_maps_a, cores)
    contrib = np.concatenate(
        [res_a.results[c]["contrib"] for c in range(NCORES)]
    )

    cs = contrib[prep["sigma"]]  # postorder permutation (host, index-only)
    triu = np.triu(np.ones((P, P), np.float32), 1)
    in_maps_b = [
        {
            "contrib": contrib[c * S : (c + 1) * S],
            "cs": cs[c * S : (c + 1) * S],
            "triu": triu,
        }
        for c in range(NCORES)
    ]
    res_b = run_bass_kernel_spmd(nc_b, in_maps_b, cores)

    # host: fold core-level offsets, sample R, combine (index glue + O(N) adds)
    p1a = np.concatenate([res_b.results[c]["p1a"] for c in range(NCORES)])
    ra = np.concatenate([res_b.results[c]["ra"] for c in range(NCORES)])
    t1 = p1a[S - 1 :: S].astype(np.float32)
    t2 = ra[S - 1 :: S].astype(np.float32)
    o1 = np.repeat(
        np.concatenate([[0], np.cumsum(t1[:-1])]).astype(np.float32), S
    )
    o2 = np.repeat(
        np.concatenate([[0], np.cumsum(t2[:-1])]).astype(np.float32), S
    )
    rg = (ra + o2).astype(np.float32)
    q = prep["q"]
    p2 = np.where(q >= 0, rg[np.maximum(q, 0)], np.float32(0.0))
    out_new = ((p1a + o1) - p2).astype(np.float32)
    return out_new, progs


def kernel(maxtree_parent, maxtree_diff, attributes, weight, bias):
    w = np.asarray(weight, dtype=np.float32)[:, 0]
    b = float(np.asarray(bias, dtype=np.float32)[0])
    in_maps_a, prep = _prepare_inputs(
        maxtree_parent, maxtree_diff, attributes
    )
    out_new, _ = _run_device(in_maps_a, prep, w, b)
    out = np.empty(N, np.float32)
    out[prep["order"]] = out_new
    return out.reshape(H, W)



# revision 2
# speedup vs baseline: 1.8701x; 1.8701x over previous
"""Trainium2 Bass kernel for nn_DifferentialMaxtree (N = 4M tree nodes), v2.

Same math as the baseline (DFS-interval identity turns the tree filter into
two prefix sums), restructured around the measured environment costs:
  - DMA costs ~8 ns per AP element (dtype-independent) -> ship all node
    streams as fp16 *packed into wide elements* (bitcast u64/f32 APs), which
    cuts DMA element counts 4-8x vs the fp32 baseline.
  - tensor_tensor_scan costs ~2.7 us/element -> the 524288-long prefix sums
    run over a partition-interleaved node layout (node i at [i%128, i//128])
    as triangular-matmul partition prefixes + a 32-step scan over
    column-group sums (PE does the O(N) work instead of the DVE recurrence).
  - ~20 us/instruction dispatch -> few, large instructions everywhere.

Pipeline (8-way data-parallel over tree nodes, one shard per NeuronCore):
  A) linear/sigmoid head over the 15 raw attributes -> contrib = diff*score
  B) combined [128, 2F] matmul-prefix of contrib (P1) and of the host
     sigma-permuted contrib (R)
Host glue between/after programs is index-space permutation only, as in the
baseline.
"""

import math
import numpy as np

N = 4194304
H = W = 2048
NCORES = 8
P = 128
S = N // NCORES  # 524288 nodes per core
F = S // P  # 4096 = free size of the [128, F] interleaved node grid
F2 = 2 * F
CG = F // P  # 32 column-group entries per partition in the offset scan
EPS = 1e-10

KT = 1024  # stage-A columns (of the [128, F] grid) per tile
NT = F // KT  # stage-A tiles

PACK = 4  # DMA element width in bytes for the fp16 node streams (u64 DMA
# elements are rejected by the runtime, so f32 packing is the widest usable)


def _pdt(mybir):
    return mybir.dt.uint64 if PACK == 8 else mybir.dt.float32


# ---- inlined compat: this walrus build rejects >1 semaphore wait per
# instruction ("Too many sync wait commands"); split extras onto nops ----
def _split_excess_waits(nc, max_waits=1):
    from concourse import mybir

    counter = 0
    for f in nc.m.functions:
        for bb in f.blocks:
            il = bb.instructions
            i = 0
            while i < len(il):
                inst = il[i]
                si = inst.sync_info
                if si is not None and len(si.on_wait) > max_waits:
                    waits = list(si.on_wait)
                    rest, keep = waits[:-max_waits], waits[-max_waits:]
                    pre = []
                    for j in range(0, len(rest), max_waits):
                        nop = mybir.InstNoOp(
                            name=f"I-waitsplit-{counter}", ins=[], outs=[]
                        )
                        nop.engine = inst.engine
                        nop.sync_info = mybir.SyncInfo(
                            on_wait=list(rest[j : j + max_waits]), on_update=[]
                        )
                        counter += 1
                        pre.append(nop)
                    inst.sync_info = mybir.SyncInfo(
                        on_wait=keep, on_update=list(si.on_update)
                    )
                    for k, p_ in enumerate(pre):
                        il.insert(i + k, p_)
                    i += len(pre)
                i += 1
    return counter


def _host_prep(parent):
    """DFS relabeling, interval ends, postorder rank and sample positions."""
    import scipy.sparse as sp
    from scipy.sparse.csgraph import depth_first_order

    parent = np.asarray(parent).astype(np.int64)
    idx = np.arange(1, N, dtype=np.int64)
    g = sp.csr_matrix((np.ones(N - 1, np.int8), (parent[1:], idx)), shape=(N, N))
    order = np.asarray(
        depth_first_order(g, 0, directed=True, return_predecessors=False),
        dtype=np.int64,
    )
    assert order.shape[0] == N, f"tree not rooted/connected: {order.shape}"

    # depth (number of proper ancestors) via pointer doubling
    SENT = N
    p = np.concatenate([parent, [SENT]])
    p[0] = SENT
    chains = []
    pk = p.copy()
    while not np.all(pk[:N] == SENT):
        chains.append(pk.copy())
        pk = pk[pk]
        pk[SENT] = SENT
    depth = np.zeros(N + 1, np.int64)
    cur = np.arange(N + 1)
    for k in range(len(chains) - 1, -1, -1):
        anc = chains[k][cur]
        mask = anc != SENT
        depth[mask] += 1 << k
        cur = np.where(mask, anc, cur)
    d_old = depth[:N]

    # subtree sizes: accumulate child -> parent, deepest level first
    size = np.ones(N, np.int64)
    dorder = np.argsort(d_old, kind="stable")
    maxd = int(d_old.max())
    dstarts = np.searchsorted(d_old[dorder], np.arange(maxd + 2))
    for lev in range(maxd, 0, -1):
        nodes = dorder[dstarts[lev] : dstarts[lev + 1]]
        np.add.at(size, parent[nodes], size[nodes])

    end_new = np.arange(N, dtype=np.int64) + size[order] - 1
    d_new = d_old[order]
    sigma = np.argsort(end_new, kind="stable")  # rank r -> source node t
    q = np.arange(N, dtype=np.int64) - d_new - 1  # P2 sample position (rho-1)
    return {"order": order, "sigma": sigma, "q": q}


def _build_stage_a(w, b, repeat=1):
    """Program A: attribute head -> contrib (fp16, interleaved node grid)."""
    from concourse import bass, mybir
    import concourse.tile as tile

    DT = mybir.dt.float32
    HT = mybir.dt.float16
    PT = _pdt(mybir)
    PE = PACK // 2  # fp16 values per packed element
    AF = mybir.ActivationFunctionType
    OP = mybir.AluOpType

    w = [float(x) for x in w]
    b = float(b)

    nc = bass.Bass()
    attr_d = nc.declare_dram_parameter("attr", [S * 15 // PE], PT, isOutput=False)
    diff_d = nc.declare_dram_parameter("diff", [S // PE], PT, isOutput=False)
    con_d = nc.declare_dram_parameter("contrib", [S // PE], PT, isOutput=True)

    # ACT biases must exist as const APs (keyed fp32 by scalar_like)
    for _cv in (0.0, math.pi / 2):
        _ct = nc.alloc_sbuf_tensor(f"const-f32-{_cv}", [P, 1], DT)
        nc.gpsimd.memset(_ct.ap(), _cv)
        nc.const_aps.aps[(DT, _cv)] = _ct.ap()
    nc.all_engine_barrier()

    KP = KT * 15 // PE  # packed attr elems per partition per tile
    DP = KT // PE  # packed diff/contrib elems per partition per tile

    with tile.TileContext(nc) as tc:
        with tc.tile_pool(name="sbuf", bufs=2) as pool:
            for _rep in range(repeat):
                # attr layout: [P, NT, 15, KT] fp16; diff/contrib: [P, NT, KT]
                attr_v = attr_d[:].rearrange("(p x) -> p x", p=P)
                diff_v = diff_d[:].rearrange("(p f) -> p f", p=P)
                con_v = con_d[:].rearrange("(p f) -> p f", p=P)
                for t in range(NT):
                    K = KT
                    at = pool.tile([P, K * 15], HT, tag="at")
                    # two DMAs so per-partition rows stay <= 16KB
                    atp = at[:].bitcast(PT)
                    hp = KP // 2
                    nc.sync.dma_start(
                        out=atp[:, 0:hp],
                        in_=attr_v[:, t * KP : t * KP + hp],
                    )
                    nc.sync.dma_start(
                        out=atp[:, hp:KP],
                        in_=attr_v[:, t * KP + hp : (t + 1) * KP],
                    )
                    dt_ = pool.tile([P, K], HT, tag="dt")
                    nc.sync.dma_start(
                        out=dt_[:].bitcast(PT),
                        in_=diff_v[:, t * DP : (t + 1) * DP],
                    )

                    # feature-major: column j is the contiguous [P, K] slab j
                    def col(j):
                        return at[:, j * K : (j + 1) * K]

                    # logs of columns 6..14 (inputs >= 1e-3, so the +EPS of
                    # the reference is numerically irrelevant)
                    feat = pool.tile([P, 9 * K], HT, tag="feat")
                    nc.scalar.activation(feat[:], at[:, 6 * K : 15 * K], AF.Ln)
                    # range-reduce angles to [0, 2pi) before the Sin LUT
                    tq = pool.tile([P, K], DT, tag="tq")
                    nc.vector.tensor_scalar_mul(
                        tq[:], col(5), 1.0 / (2 * math.pi)
                    )
                    tqi = pool.tile([P, K], mybir.dt.int32, tag="tqi")
                    nc.vector.tensor_copy(out=tqi[:], in_=tq[:])
                    nc.vector.tensor_copy(out=tq[:], in_=tqi[:])
                    ang = pool.tile([P, K], DT, tag="ang")
                    nc.vector.scalar_tensor_tensor(
                        out=ang[:], in0=tq[:], scalar=-2 * math.pi,
                        in1=col(5), op0=OP.mult, op1=OP.add,
                    )
                    cosv = pool.tile([P, K], HT, tag="cosv")
                    nc.scalar.activation(cosv[:], ang[:], AF.Sin, bias=math.pi / 2)
                    sinv = pool.tile([P, K], HT, tag="sinv")
                    nc.scalar.activation(sinv[:], ang[:], AF.Sin)
                    # lshape = sqrt(x7)/(sqrt(x6)+EPS) == sqrt(x7/x6) to 3e-9
                    rec = pool.tile([P, K], DT, tag="rec")
                    nc.vector.reciprocal(rec[:], col(6))
                    rat = pool.tile([P, K], DT, tag="rat")
                    nc.vector.tensor_tensor(
                        out=rat[:], in0=rec[:], in1=col(7), op=OP.mult
                    )
                    lsh = pool.tile([P, K], DT, tag="lsh")
                    nc.scalar.activation(lsh[:], rat[:], AF.Sqrt)

                    # weighted sum: two independent accumulator chains
                    y = pool.tile([P, K], DT, tag="y")
                    nc.scalar.activation(y[:], col(0), AF.Copy, bias=b, scale=w[0])
                    y1 = pool.tile([P, K], DT, tag="y1")
                    nc.vector.tensor_scalar(
                        out=y1[:], in0=col(1), scalar1=w[1], scalar2=None,
                        op0=OP.mult,
                    )
                    terms = (
                        [(col(j), w[j]) for j in range(2, 5)]
                        + [
                            (feat[:, j * K : (j + 1) * K], w[5 + j])
                            for j in range(9)
                        ]
                        + [(lsh[:], w[14]), (cosv[:], w[15]), (sinv[:], w[16])]
                    )
                    for n, (src, wt) in enumerate(terms):
                        acc = y if n % 2 == 0 else y1
                        nc.vector.scalar_tensor_tensor(
                            out=acc[:], in0=src, scalar=wt, in1=acc[:],
                            op0=OP.mult, op1=OP.add,
                        )
                    nc.vector.tensor_tensor(
                        out=y[:], in0=y[:], in1=y1[:], op=OP.add
                    )
                    sc = pool.tile([P, K], DT, tag="sc")
                    nc.scalar.activation(sc[:], y[:], AF.Sigmoid)
                    ct = pool.tile([P, K], HT, tag="ct")
                    nc.vector.tensor_tensor(
                        out=ct[:], in0=sc[:], in1=dt_[:], op=OP.mult
                    )
                    nc.sync.dma_start(
                        out=con_v[:, t * DP : (t + 1) * DP],
                        in_=ct[:].bitcast(PT),
                    )

    _split_excess_waits(nc)
    return nc


def _build_scans(repeat=1):
    """Program B: combined inclusive prefix sums of contrib (P1) and of the
    sigma-permuted contrib (R), p-major node layout.

    Input  cc [P, 2F] fp16 (packed): cols 0..F-1 = contrib, F..2F-1 = cs.
    Output pr [P, 2F] fp32: cols 0..F-1 = P1, F..2F-1 = R.
    Node i of a stream lives at [i // F, i % F].

    In-row prefixes run as a Hillis-Steele doubling ladder on the DVE (big
    tensor_tensor adds at ~57 ns/elem/partition beat tensor_tensor_scan's
    ~2.7 us/elem serial recurrence by ~40x); cross-partition row offsets
    come from one strict-triangular matmul.
    """
    from concourse import bass, mybir
    import concourse.tile as tile

    DT = mybir.dt.float32
    PT = _pdt(mybir)
    PE = PACK // 2
    OP = mybir.AluOpType

    STEPS = 12  # log2(F)
    assert 1 << STEPS == F

    nc = bass.Bass()
    cc_d = nc.declare_dram_parameter("cc", [P * F2 // PE], PT, isOutput=False)
    stri_d = nc.declare_dram_parameter("stri", [P, P], DT, isOutput=False)  # k<m
    pr_d = nc.declare_dram_parameter("pr", [P * F2], DT, isOutput=True)

    HT = mybir.dt.float16

    with tile.TileContext(nc) as tc:
        with (
            tc.tile_pool(name="perm", bufs=1) as perm,
            tc.tile_pool(name="psum", bufs=2, space="PSUM") as psum,
        ):
            stri = perm.tile([P, P], DT)
            nc.sync.dma_start(out=stri[:], in_=stri_d[:])
            for _rep in range(repeat):
                cc = perm.tile([P, F2], HT, tag="cc")
                ccp = cc[:].bitcast(PT)
                ccv = cc_d[:].rearrange("(p f) -> p f", p=P)
                hc = F2 // PE // 2
                nc.sync.dma_start(out=ccp[:, 0:hc], in_=ccv[:, 0:hc])
                nc.sync.dma_start(
                    out=ccp[:, hc : 2 * hc], in_=ccv[:, hc : 2 * hc]
                )
                bufa = perm.tile([P, F2], DT, tag="bufa")
                bufb = perm.tile([P, F2], DT, tag="bufb")
                src = cc
                dst = bufa
                for step in range(STEPS):
                    k = 1 << step
                    for s in range(2):
                        o = s * F
                        nc.vector.tensor_tensor(
                            out=dst[:, o + k : o + F],
                            in0=src[:, o + k : o + F],
                            in1=src[:, o : o + F - k],
                            op=OP.add,
                        )
                        nc.vector.tensor_copy(
                            out=dst[:, o : o + k], in_=src[:, o : o + k]
                        )
                    src, dst = dst, (bufb if dst is bufa else bufa)
                # src now holds the in-row inclusive prefixes
                rt = perm.tile([P, 2], DT, tag="rt")
                nc.vector.tensor_copy(out=rt[:, 0:1], in_=src[:, F - 1 : F])
                nc.vector.tensor_copy(out=rt[:, 1:2], in_=src[:, F2 - 1 : F2])
                rop = psum.tile([P, 2], DT, tag="rop")
                nc.tensor.matmul(
                    out=rop[:], lhsT=stri[:], rhs=rt[:], start=True, stop=True
                )
                roff = perm.tile([P, 2], DT, tag="roff")
                nc.vector.tensor_copy(out=roff[:], in_=rop[:])
                out_t = dst  # the unused ping-pong buffer
                for s in range(2):
                    o = s * F
                    nc.vector.tensor_scalar(
                        out=out_t[:, o : o + F], in0=src[:, o : o + F],
                        scalar1=roff[:, s : s + 1], scalar2=None, op0=OP.add,
                    )
                prv = pr_d[:].rearrange("(p f) -> p f", p=P)
                qc = F2 // 4
                for h in range(4):
                    nc.sync.dma_start(
                        out=prv[:, h * qc : (h + 1) * qc],
                        in_=out_t[:, h * qc : (h + 1) * qc],
                    )

    _split_excess_waits(nc)
    return nc


def _pack_np(a16):
    """fp16 array (last axis contiguous) -> packed wide-element array."""
    a16 = np.ascontiguousarray(a16)
    return a16.view(np.uint64 if PACK == 8 else np.float32)


def _prepare_inputs(maxtree_parent, maxtree_diff, attributes):
    diff = np.asarray(maxtree_diff, dtype=np.float32)
    attrs = np.asarray(attributes, dtype=np.float32)
    prep = _host_prep(maxtree_parent)
    order = prep["order"]
    attr_p = attrs[order].astype(np.float16)
    diff_p = diff[order].astype(np.float16)
    # p-major node grid: core-local node j -> [j // F, j % F];
    # stage-A tile layout [P, NT, 15, KT] (feature-major slabs per tile)
    in_maps_a = []
    for c in range(NCORES):
        a = attr_p[c * S : (c + 1) * S].reshape(P, NT, KT, 15)
        a = np.ascontiguousarray(a.transpose(0, 1, 3, 2))  # [P, NT, 15, KT]
        d = diff_p[c * S : (c + 1) * S].reshape(P, F)
        in_maps_a.append(
            {
                "attr": _pack_np(a.reshape(-1)),
                "diff": _pack_np(np.ascontiguousarray(d).reshape(-1)),
            }
        )
    return in_maps_a, prep


def _run_device(in_maps_a, prep, w, b, repeat=1, progs=None):
    """Run both device programs; host applies the index permutations between
    them.  Returns (out_new, progs) where progs can be reused for re-runs."""
    from concourse.bass_utils import run_bass_kernel_spmd

    cores = list(range(NCORES))
    if progs is None:
        progs = (_build_stage_a(w, b, repeat), _build_scans(repeat))
    nc_a, nc_b = progs

    res_a = run_bass_kernel_spmd(nc_a, in_maps_a, cores)
    # p-major grid [P, F]: flatten is already linear node order
    conbuf = [
        np.ascontiguousarray(res_a.results[c]["contrib"])
        .view(np.float16)
        .reshape(P, F)
        for c in range(NCORES)
    ]
    contrib = np.concatenate([cb.reshape(-1) for cb in conbuf])

    cs = contrib[prep["sigma"]]  # postorder permutation (host, index-only)
    stri = (np.arange(P)[:, None] < np.arange(P)[None, :]).astype(np.float32)
    in_maps_b = []
    for c in range(NCORES):
        cc = np.empty((P, F2), np.float16)
        cc[:, 0:F] = conbuf[c]
        cc[:, F:F2] = cs[c * S : (c + 1) * S].reshape(P, F)
        in_maps_b.append({"cc": _pack_np(cc).reshape(-1), "stri": stri})
    res_b = run_bass_kernel_spmd(nc_b, in_maps_b, cores)

    # host: fold core-level offsets, sample R, combine (index glue + O(N) adds)
    p1a = np.empty(N, np.float32)
    ra = np.empty(N, np.float32)
    for c in range(NCORES):
        pr = np.ascontiguousarray(res_b.results[c]["pr"]).reshape(P, F2)
        p1a[c * S : (c + 1) * S] = pr[:, 0:F].reshape(-1)
        ra[c * S : (c + 1) * S] = pr[:, F:F2].reshape(-1)
    t1 = p1a[S - 1 :: S].astype(np.float32)
    t2 = ra[S - 1 :: S].astype(np.float32)
    o1 = np.repeat(
        np.concatenate([[0], np.cumsum(t1[:-1])]).astype(np.float32), S
    )
    o2 = np.repeat(
        np.concatenate([[0], np.cumsum(t2[:-1])]).astype(np.float32), S
    )
    rg = (ra + o2).astype(np.float32)
    q = prep["q"]
    p2 = np.where(q >= 0, rg[np.maximum(q, 0)], np.float32(0.0))
    out_new = ((p1a + o1) - p2).astype(np.float32)
    return out_new, progs


def kernel(maxtree_parent, maxtree_diff, attributes, weight, bias):
    w = np.asarray(weight, dtype=np.float32)[:, 0]
    b = float(np.asarray(bias, dtype=np.float32)[0])
    in_maps_a, prep = _prepare_inputs(
        maxtree_parent, maxtree_diff, attributes
    )
    out_new, _ = _run_device(in_maps_a, prep, w, b)
    out = np.empty(N, np.float32)
    out[prep["order"]] = out_new
    return out.reshape(H, W)


# revision 3
# speedup vs baseline: 2.0705x; 1.1072x over previous
"""Trainium2 Bass kernel for nn_DifferentialMaxtree (N = 4M tree nodes), v2.

Same math as the baseline (DFS-interval identity turns the tree filter into
two prefix sums), restructured around the measured environment costs:
  - DMA costs ~8 ns per AP element (dtype-independent) -> ship all node
    streams as fp16 *packed into wide elements* (bitcast u64/f32 APs), which
    cuts DMA element counts 4-8x vs the fp32 baseline.
  - tensor_tensor_scan costs ~2.7 us/element -> the 524288-long prefix sums
    run over a partition-interleaved node layout (node i at [i%128, i//128])
    as triangular-matmul partition prefixes + a 32-step scan over
    column-group sums (PE does the O(N) work instead of the DVE recurrence).
  - ~20 us/instruction dispatch -> few, large instructions everywhere.

Pipeline (8-way data-parallel over tree nodes, one shard per NeuronCore):
  A) linear/sigmoid head over the 15 raw attributes -> contrib = diff*score
  B) combined [128, 2F] matmul-prefix of contrib (P1) and of the host
     sigma-permuted contrib (R)
Host glue between/after programs is index-space permutation only, as in the
baseline.
"""

import math
import numpy as np

N = 4194304
H = W = 2048
NCORES = 8
P = 128
S = N // NCORES  # 524288 nodes per core
F = S // P  # 4096 = free size of the [128, F] interleaved node grid
F2 = 2 * F
CG = F // P  # 32 column-group entries per partition in the offset scan
EPS = 1e-10

KT = 1024  # stage-A columns (of the [128, F] grid) per tile
NT = F // KT  # stage-A tiles

PACK = 4  # DMA element width in bytes for the fp16 node streams (u64 DMA
# elements are rejected by the runtime, so f32 packing is the widest usable)


def _pdt(mybir):
    return mybir.dt.uint64 if PACK == 8 else mybir.dt.float32


# ---- inlined compat: this walrus build rejects >1 semaphore wait per
# instruction ("Too many sync wait commands"); split extras onto nops ----
def _split_excess_waits(nc, max_waits=1):
    from concourse import mybir

    counter = 0
    for f in nc.m.functions:
        for bb in f.blocks:
            il = bb.instructions
            i = 0
            while i < len(il):
                inst = il[i]
                si = inst.sync_info
                if si is not None and len(si.on_wait) > max_waits:
                    waits = list(si.on_wait)
                    rest, keep = waits[:-max_waits], waits[-max_waits:]
                    pre = []
                    for j in range(0, len(rest), max_waits):
                        nop = mybir.InstNoOp(
                            name=f"I-waitsplit-{counter}", ins=[], outs=[]
                        )
                        nop.engine = inst.engine
                        nop.sync_info = mybir.SyncInfo(
                            on_wait=list(rest[j : j + max_waits]), on_update=[]
                        )
                        counter += 1
                        pre.append(nop)
                    inst.sync_info = mybir.SyncInfo(
                        on_wait=keep, on_update=list(si.on_update)
                    )
                    for k, p_ in enumerate(pre):
                        il.insert(i + k, p_)
                    i += len(pre)
                i += 1
    return counter


def _host_prep(parent):
    """DFS relabeling, interval ends, postorder rank and sample positions."""
    import scipy.sparse as sp
    from scipy.sparse.csgraph import depth_first_order

    parent = np.asarray(parent).astype(np.int64)
    idx = np.arange(1, N, dtype=np.int64)
    g = sp.csr_matrix((np.ones(N - 1, np.int8), (parent[1:], idx)), shape=(N, N))
    order = np.asarray(
        depth_first_order(g, 0, directed=True, return_predecessors=False),
        dtype=np.int64,
    )
    assert order.shape[0] == N, f"tree not rooted/connected: {order.shape}"

    # depth (number of proper ancestors) via pointer doubling
    SENT = N
    p = np.concatenate([parent, [SENT]])
    p[0] = SENT
    chains = []
    pk = p.copy()
    while not np.all(pk[:N] == SENT):
        chains.append(pk.copy())
        pk = pk[pk]
        pk[SENT] = SENT
    depth = np.zeros(N + 1, np.int64)
    cur = np.arange(N + 1)
    for k in range(len(chains) - 1, -1, -1):
        anc = chains[k][cur]
        mask = anc != SENT
        depth[mask] += 1 << k
        cur = np.where(mask, anc, cur)
    d_old = depth[:N]

    # subtree sizes: accumulate child -> parent, deepest level first
    size = np.ones(N, np.int64)
    dorder = np.argsort(d_old, kind="stable")
    maxd = int(d_old.max())
    dstarts = np.searchsorted(d_old[dorder], np.arange(maxd + 2))
    for lev in range(maxd, 0, -1):
        nodes = dorder[dstarts[lev] : dstarts[lev + 1]]
        np.add.at(size, parent[nodes], size[nodes])

    end_new = np.arange(N, dtype=np.int64) + size[order] - 1
    d_new = d_old[order]
    sigma = np.argsort(end_new, kind="stable")  # rank r -> source node t
    q = np.arange(N, dtype=np.int64) - d_new - 1  # P2 sample position (rho-1)
    return {"order": order, "sigma": sigma, "q": q}


def _build_stage_a(w, b, repeat=1):
    """Program A: attribute head -> contrib (fp16, interleaved node grid)."""
    from concourse import bass, mybir
    import concourse.tile as tile

    DT = mybir.dt.float32
    HT = mybir.dt.float16
    PT = _pdt(mybir)
    PE = PACK // 2  # fp16 values per packed element
    AF = mybir.ActivationFunctionType
    OP = mybir.AluOpType

    w = [float(x) for x in w]
    b = float(b)

    nc = bass.Bass()
    attr_d = nc.declare_dram_parameter("attr", [S * 15 // PE], PT, isOutput=False)
    diff_d = nc.declare_dram_parameter("diff", [S // PE], PT, isOutput=False)
    con_d = nc.declare_dram_parameter("contrib", [S // PE], PT, isOutput=True)

    # ACT biases must exist as const APs (keyed fp32 by scalar_like)
    for _cv in (0.0, math.pi / 2):
        _ct = nc.alloc_sbuf_tensor(f"const-f32-{_cv}", [P, 1], DT)
        nc.gpsimd.memset(_ct.ap(), _cv)
        nc.const_aps.aps[(DT, _cv)] = _ct.ap()
    nc.all_engine_barrier()

    KP = KT * 15 // PE  # packed attr elems per partition per tile
    DP = KT // PE  # packed diff/contrib elems per partition per tile

    with tile.TileContext(nc) as tc:
        with tc.tile_pool(name="sbuf", bufs=2) as pool:
            for _rep in range(repeat):
                # attr layout: [P, NT, 15, KT] fp16; diff/contrib: [P, NT, KT]
                attr_v = attr_d[:].rearrange("(p x) -> p x", p=P)
                diff_v = diff_d[:].rearrange("(p f) -> p f", p=P)
                con_v = con_d[:].rearrange("(p f) -> p f", p=P)
                for t in range(NT):
                    K = KT
                    at = pool.tile([P, K * 15], HT, tag="at")
                    # two DMAs so per-partition rows stay <= 16KB
                    atp = at[:].bitcast(PT)
                    hp = KP // 2
                    nc.sync.dma_start(
                        out=atp[:, 0:hp],
                        in_=attr_v[:, t * KP : t * KP + hp],
                    )
                    nc.sync.dma_start(
                        out=atp[:, hp:KP],
                        in_=attr_v[:, t * KP + hp : (t + 1) * KP],
                    )
                    dt_ = pool.tile([P, K], HT, tag="dt")
                    nc.sync.dma_start(
                        out=dt_[:].bitcast(PT),
                        in_=diff_v[:, t * DP : (t + 1) * DP],
                    )

                    # feature-major: column j is the contiguous [P, K] slab j
                    def col(j):
                        return at[:, j * K : (j + 1) * K]

                    # logs of columns 6..14 (inputs >= 1e-3, so the +EPS of
                    # the reference is numerically irrelevant)
                    feat = pool.tile([P, 9 * K], HT, tag="feat")
                    nc.scalar.activation(feat[:], at[:, 6 * K : 15 * K], AF.Ln)
                    # range-reduce angles to [0, 2pi) before the Sin LUT
                    tq = pool.tile([P, K], DT, tag="tq")
                    nc.vector.tensor_scalar_mul(
                        tq[:], col(5), 1.0 / (2 * math.pi)
                    )
                    tqi = pool.tile([P, K], mybir.dt.int32, tag="tqi")
                    nc.vector.tensor_copy(out=tqi[:], in_=tq[:])
                    nc.vector.tensor_copy(out=tq[:], in_=tqi[:])
                    ang = pool.tile([P, K], DT, tag="ang")
                    nc.vector.scalar_tensor_tensor(
                        out=ang[:], in0=tq[:], scalar=-2 * math.pi,
                        in1=col(5), op0=OP.mult, op1=OP.add,
                    )
                    cosv = pool.tile([P, K], HT, tag="cosv")
                    nc.scalar.activation(cosv[:], ang[:], AF.Sin, bias=math.pi / 2)
                    sinv = pool.tile([P, K], HT, tag="sinv")
                    nc.scalar.activation(sinv[:], ang[:], AF.Sin)
                    # lshape = sqrt(x7)/(sqrt(x6)+EPS) == sqrt(x7/x6) to 3e-9
                    rec = pool.tile([P, K], DT, tag="rec")
                    nc.vector.reciprocal(rec[:], col(6))
                    rat = pool.tile([P, K], DT, tag="rat")
                    nc.vector.tensor_tensor(
                        out=rat[:], in0=rec[:], in1=col(7), op=OP.mult
                    )
                    lsh = pool.tile([P, K], DT, tag="lsh")
                    nc.scalar.activation(lsh[:], rat[:], AF.Sqrt)

                    # weighted sum: two independent accumulator chains
                    # (fp16 accumulators: 16-bit DVE ops run at 2x; the
                    # rounding adds ~2e-4 score error, far under tolerance)
                    y = pool.tile([P, K], HT, tag="y")
                    nc.scalar.activation(y[:], col(0), AF.Copy, bias=b, scale=w[0])
                    y1 = pool.tile([P, K], HT, tag="y1")
                    nc.vector.tensor_scalar(
                        out=y1[:], in0=col(1), scalar1=w[1], scalar2=None,
                        op0=OP.mult,
                    )
                    terms = (
                        [(col(j), w[j]) for j in range(2, 5)]
                        + [
                            (feat[:, j * K : (j + 1) * K], w[5 + j])
                            for j in range(9)
                        ]
                        + [(lsh[:], w[14]), (cosv[:], w[15]), (sinv[:], w[16])]
                    )
                    for n, (src, wt) in enumerate(terms):
                        acc = y if n % 2 == 0 else y1
                        nc.vector.scalar_tensor_tensor(
                            out=acc[:], in0=src, scalar=wt, in1=acc[:],
                            op0=OP.mult, op1=OP.add,
                        )
                    nc.vector.tensor_tensor(
                        out=y[:], in0=y[:], in1=y1[:], op=OP.add
                    )
                    sc = pool.tile([P, K], HT, tag="sc")
                    nc.scalar.activation(sc[:], y[:], AF.Sigmoid)
                    ct = pool.tile([P, K], HT, tag="ct")
                    nc.vector.tensor_tensor(
                        out=ct[:], in0=sc[:], in1=dt_[:], op=OP.mult
                    )
                    nc.sync.dma_start(
                        out=con_v[:, t * DP : (t + 1) * DP],
                        in_=ct[:].bitcast(PT),
                    )

    _split_excess_waits(nc)
    return nc


def _build_scans(repeat=1):
    """Program B: combined inclusive prefix sums of contrib (P1) and of the
    sigma-permuted contrib (R), p-major node layout.

    Input  cc [P, 2F] fp16 (packed): cols 0..F-1 = contrib, F..2F-1 = cs.
    Output pr [P, 2F] fp32: cols 0..F-1 = P1, F..2F-1 = R.
    Node i of a stream lives at [i // F, i % F].

    In-row prefixes run as a Hillis-Steele doubling ladder on the DVE (big
    tensor_tensor adds at ~57 ns/elem/partition beat tensor_tensor_scan's
    ~2.7 us/elem serial recurrence by ~40x); cross-partition row offsets
    come from one strict-triangular matmul.
    """
    from concourse import bass, mybir
    import concourse.tile as tile

    DT = mybir.dt.float32
    PT = _pdt(mybir)
    PE = PACK // 2
    OP = mybir.AluOpType

    STEPS = 12  # log2(F)
    assert 1 << STEPS == F

    nc = bass.Bass()
    cc_d = nc.declare_dram_parameter("cc", [P * F2 // PE], PT, isOutput=False)
    stri_d = nc.declare_dram_parameter("stri", [P, P], DT, isOutput=False)  # k<m
    pr_d = nc.declare_dram_parameter("pr", [P * F2], DT, isOutput=True)

    HT = mybir.dt.float16

    with tile.TileContext(nc) as tc:
        with (
            tc.tile_pool(name="perm", bufs=1) as perm,
            tc.tile_pool(name="psum", bufs=2, space="PSUM") as psum,
        ):
            stri = perm.tile([P, P], DT)
            nc.sync.dma_start(out=stri[:], in_=stri_d[:])
            for _rep in range(repeat):
                cc = perm.tile([P, F2], HT, tag="cc")
                ccp = cc[:].bitcast(PT)
                ccv = cc_d[:].rearrange("(p f) -> p f", p=P)
                hc = F2 // PE // 2
                nc.sync.dma_start(out=ccp[:, 0:hc], in_=ccv[:, 0:hc])
                nc.sync.dma_start(
                    out=ccp[:, hc : 2 * hc], in_=ccv[:, hc : 2 * hc]
                )
                bufa = perm.tile([P, F2], DT, tag="bufa")
                bufb = perm.tile([P, F2], DT, tag="bufb")
                src = cc
                dst = bufa
                for step in range(STEPS):
                    k = 1 << step
                    sv = src[:].rearrange("p (s f) -> p s f", s=2)
                    dv = dst[:].rearrange("p (s f) -> p s f", s=2)
                    nc.vector.tensor_tensor(
                        out=dv[:, :, k:F], in0=sv[:, :, k:F],
                        in1=sv[:, :, 0 : F - k], op=OP.add,
                    )
                    nc.vector.tensor_copy(
                        out=dv[:, :, 0:k], in_=sv[:, :, 0:k]
                    )
                    src, dst = dst, (bufb if dst is bufa else bufa)
                # src now holds the in-row inclusive prefixes
                rt = perm.tile([P, 2], DT, tag="rt")
                nc.vector.tensor_copy(out=rt[:, 0:1], in_=src[:, F - 1 : F])
                nc.vector.tensor_copy(out=rt[:, 1:2], in_=src[:, F2 - 1 : F2])
                rop = psum.tile([P, 2], DT, tag="rop")
                nc.tensor.matmul(
                    out=rop[:], lhsT=stri[:], rhs=rt[:], start=True, stop=True
                )
                roff = perm.tile([P, 2], DT, tag="roff")
                nc.vector.tensor_copy(out=roff[:], in_=rop[:])
                out_t = dst  # the unused ping-pong buffer
                for s in range(2):
                    o = s * F
                    nc.vector.tensor_scalar(
                        out=out_t[:, o : o + F], in0=src[:, o : o + F],
                        scalar1=roff[:, s : s + 1], scalar2=None, op0=OP.add,
                    )
                prv = pr_d[:].rearrange("(p f) -> p f", p=P)
                qc = F2 // 4
                for h in range(4):
                    nc.sync.dma_start(
                        out=prv[:, h * qc : (h + 1) * qc],
                        in_=out_t[:, h * qc : (h + 1) * qc],
                    )

    _split_excess_waits(nc)
    return nc


def _pack_np(a16):
    """fp16 array (last axis contiguous) -> packed wide-element array."""
    a16 = np.ascontiguousarray(a16)
    return a16.view(np.uint64 if PACK == 8 else np.float32)


def _prepare_inputs(maxtree_parent, maxtree_diff, attributes):
    diff = np.asarray(maxtree_diff, dtype=np.float32)
    attrs = np.asarray(attributes, dtype=np.float32)
    prep = _host_prep(maxtree_parent)
    order = prep["order"]
    attr_p = attrs[order].astype(np.float16)
    diff_p = diff[order].astype(np.float16)
    # p-major node grid: core-local node j -> [j // F, j % F];
    # stage-A tile layout [P, NT, 15, KT] (feature-major slabs per tile)
    in_maps_a = []
    for c in range(NCORES):
        a = attr_p[c * S : (c + 1) * S].reshape(P, NT, KT, 15)
        a = np.ascontiguousarray(a.transpose(0, 1, 3, 2))  # [P, NT, 15, KT]
        d = diff_p[c * S : (c + 1) * S].reshape(P, F)
        in_maps_a.append(
            {
                "attr": _pack_np(a.reshape(-1)),
                "diff": _pack_np(np.ascontiguousarray(d).reshape(-1)),
            }
        )
    return in_maps_a, prep


def _run_device(in_maps_a, prep, w, b, repeat=1, progs=None):
    """Run both device programs; host applies the index permutations between
    them.  Returns (out_new, progs) where progs can be reused for re-runs."""
    from concourse.bass_utils import run_bass_kernel_spmd

    cores = list(range(NCORES))
    if progs is None:
        progs = (_build_stage_a(w, b, repeat), _build_scans(repeat))
    nc_a, nc_b = progs

    res_a = run_bass_kernel_spmd(nc_a, in_maps_a, cores)
    # p-major grid [P, F]: flatten is already linear node order
    conbuf = [
        np.ascontiguousarray(res_a.results[c]["contrib"])
        .view(np.float16)
        .reshape(P, F)
        for c in range(NCORES)
    ]
    contrib = np.concatenate([cb.reshape(-1) for cb in conbuf])

    cs = contrib[prep["sigma"]]  # postorder permutation (host, index-only)
    stri = (np.arange(P)[:, None] < np.arange(P)[None, :]).astype(np.float32)
    in_maps_b = []
    for c in range(NCORES):
        cc = np.empty((P, F2), np.float16)
        cc[:, 0:F] = conbuf[c]
        cc[:, F:F2] = cs[c * S : (c + 1) * S].reshape(P, F)
        in_maps_b.append({"cc": _pack_np(cc).reshape(-1), "stri": stri})
    res_b = run_bass_kernel_spmd(nc_b, in_maps_b, cores)

    # host: fold core-level offsets, sample R, combine (index glue + O(N) adds)
    p1a = np.empty(N, np.float32)
    ra = np.empty(N, np.float32)
    for c in range(NCORES):
        pr = np.ascontiguousarray(res_b.results[c]["pr"]).reshape(P, F2)
        p1a[c * S : (c + 1) * S] = pr[:, 0:F].reshape(-1)
        ra[c * S : (c + 1) * S] = pr[:, F:F2].reshape(-1)
    t1 = p1a[S - 1 :: S].astype(np.float32)
    t2 = ra[S - 1 :: S].astype(np.float32)
    o1 = np.repeat(
        np.concatenate([[0], np.cumsum(t1[:-1])]).astype(np.float32), S
    )
    o2 = np.repeat(
        np.concatenate([[0], np.cumsum(t2[:-1])]).astype(np.float32), S
    )
    rg = (ra + o2).astype(np.float32)
    q = prep["q"]
    p2 = np.where(q >= 0, rg[np.maximum(q, 0)], np.float32(0.0))
    out_new = ((p1a + o1) - p2).astype(np.float32)
    return out_new, progs


def kernel(maxtree_parent, maxtree_diff, attributes, weight, bias):
    w = np.asarray(weight, dtype=np.float32)[:, 0]
    b = float(np.asarray(bias, dtype=np.float32)[0])
    in_maps_a, prep = _prepare_inputs(
        maxtree_parent, maxtree_diff, attributes
    )
    out_new, _ = _run_device(in_maps_a, prep, w, b)
    out = np.empty(N, np.float32)
    out[prep["order"]] = out_new
    return out.reshape(H, W)
